# revision 12
# baseline (speedup 1.0000x reference)
"""MultiHeadPool Trainium2 kernel (bf16, transpose-free).

Per-core computation (batch b of 8, one per NeuronCore):
  X = others[b]          (N=64, T=512, D=128)
  L = X . qT * scale     contraction over d   -> (T, H, N) logits
  W = softmax_n(L)
  ctx = W . X            contraction over n   -> (T, H, D)

The kernel is PE-instruction-issue bound (~70-80ns/instr), so the design
minimizes Tensor-engine instruction count:
  - bf16 operands: 1 LDWEIGHTS + 1 MATMUL per logical matmul (fp32 emits
    2+2), and half the HBM traffic.
  - The host sends TWO layouts of X: xd[d, (t n)] (d-major) and
    xjn[(j n), tp, d'] (pair-major, d'=D plus a ones column). mm1 loads
    its stationary X^T_c directly from xd -- no PE transposes, no
    PSUM->SBUF relay copies.
  - Block-diagonal zero padding of the E tiles is written ONCE into a
    persistent 2-slot tile; per-chunk exp writes only the diagonal
    blocks.

Per t-pair c (rows = 64j+n, two timesteps t=2c+j):
  mm1: L_c[(jn), h] = xd[:,128c:128c+128].T @ qt      (PSUM, f=7)
  exp: E[(jn), c, j'*7+h] = exp(L) on the j==j' diagonal blocks (bf16)
  mm2: ctx-pair = E_c.T @ xjn_c -> (32, 129) at col-group 32*g2;
       col 128 = softmax denominators; 4 col-groups packed on the PE
  DVE: reciprocal + broadcast scale; DMA out (host reassembles order)
"""

import sys

for p in ("/opt/trn_rl_repo", "/root/.axon_site/_ro/trn_rl_repo"):
    if p not in sys.path:
        sys.path.append(p)

from contextlib import ExitStack

import numpy as np
import ml_dtypes

import concourse.bacc as bacc
import concourse.bass as bass
import concourse.tile as tile
from concourse import mybir
from concourse.bass_utils import run_bass_kernel_spmd

B, N, T, D, H = 8, 64, 512, 128, 7
CH = 16               # t-pairs per chunk
NG = (T // 2) // CH   # 16 chunks per batch
F32 = mybir.dt.float32
BF16 = mybir.dt.bfloat16
BF16_NP = ml_dtypes.bfloat16

_CACHE = {}


def _body(ctx, tc, xd, xjn, qt, ob, repeat=1):
    nc = tc.nc
    E2 = 2 * H            # 14 data cols; pad to 32 for full g2 row groups
    M2 = 32
    DE = D + 1            # 129
    NSLOT = 2             # e-tile slots (manual rotation, zeros persist)

    singles = ctx.enter_context(tc.tile_pool(name="singles", bufs=1))
    ltp = ctx.enter_context(tc.tile_pool(name="ltp", bufs=1, space="PSUM"))
    ctxp = ctx.enter_context(tc.tile_pool(name="ctxp", bufs=3, space="PSUM"))
    rp = ctx.enter_context(tc.tile_pool(name="rp", bufs=4))
    stg = ctx.enter_context(tc.tile_pool(name="stg", bufs=3))

    qt_sb = singles.tile([D, H], BF16)
    nc.sync.dma_start(out=qt_sb[:], in_=qt[:])

    # persistent SBUF-resident inputs; chunk DMAs are issued with a small
    # prefetch depth so output DMAs (scalar queue) interleave with the
    # input stream instead of queueing behind all of it
    xd_sb = singles.tile([128, NG, CH * 128], BF16)
    xjn_sb = singles.tile([128, T // 2, DE], BF16)
    PF = 3

    def fetch(g):
        # xd and xjn on separate DMA dispatch queues (SP / Activation) so
        # the two input streams transfer concurrently
        nc.sync.dma_start(out=xd_sb[:, g, :], in_=xd[:, g, :])
        nc.scalar.dma_start(out=xjn_sb[:, CH * g: CH * (g + 1), :],
                            in_=xjn[:, CH * g: CH * (g + 1), :])

    for g in range(min(PF, NG)):
        fetch(g)

    # E tiles: [jn, slot, c, m]; m in 0:7 is the j=0 block, 7:14 the j=1
    # block, 14:32 zero pad (keeps mm2 output a full 32-row group). The
    # off-diagonal + pad zeros are written once and never touched again.
    e_all = singles.tile([128, NSLOT, CH, M2], BF16)
    nc.vector.memset(e_all[:], 0.0)

    # one persistent L bank with 4 rotating slots (448 fp32 <= 1 bank);
    # slot-reuse deps are 4 chunks old -> elided
    ltall = ltp.tile([128, 4, CH, H], F32)

    for g in range(NG * repeat):
        g = g % NG
        if g + PF < NG:
            fetch(g + PF)
        ltb = ltall[:, g % 4]
        for i in range(CH):
            nc.tensor.matmul(
                ltb[:, i, :],
                lhsT=xd_sb[:, g, 128 * i: 128 * (i + 1)],
                rhs=qt_sb[:],
                start=True, stop=True,
            )

        e_g = e_all[:, g % NSLOT]
        nc.scalar.activation(
            out=e_g[0:64, :, 0:H], in_=ltb[0:64],
            func=mybir.ActivationFunctionType.Exp,
        )
        nc.scalar.activation(
            out=e_g[64:128, :, H:E2], in_=ltb[64:128],
            func=mybir.ActivationFunctionType.Exp,
        )

        # mm2: 8 pairs per PSUM bank; col-groups iterate fastest so the
        # four 32-col PE tiles run concurrently
        stc = stg.tile([128, 2, 2, D], BF16)
        for half in range(2):
            ctxb = ctxp.tile([128, 2, DE], F32)
            for k in range(2):
                for g2 in range(4):
                    c = half * 8 + 2 * g2 + k
                    nc.tensor.matmul(
                        ctxb[32 * g2: 32 * (g2 + 1), k, :],
                        lhsT=e_g[:, c, :],
                        rhs=xjn_sb[:, CH * g + c, :],
                        start=True, stop=True,
                        tile_position=(0, 32 * g2),
                    )
            rr = rp.tile([128, 2], F32)
            nc.vector.reciprocal(rr[:], ctxb[:, :, D])
            for k in range(2):
                nc.vector.tensor_scalar_mul(
                    stc[:, half, k, :], ctxb[:, k, 0:D], rr[:, k: k + 1],
                )
        # one store per chunk, interleaved on the sync queue between input
        # chunk fetches so outputs never queue behind the whole input stream
        nc.sync.dma_start(out=ob[g], in_=stc[:])


def _build(repeat=1):
    # Bacc (not bare Bass): its compile() runs move_matmul_waits_to_ldweights
    # + generate_event_semaphores, which legalize multi-wait instructions for
    # the TRN2 one-wait-per-instruction constraint.
    nc = bacc.Bacc("TRN2", target_bir_lowering=False, debug=False)
    xd = nc.dram_tensor("xd", [128, NG, CH * 128], BF16, kind="ExternalInput")
    xjn = nc.dram_tensor("xjn", [128, T // 2, D + 1], BF16, kind="ExternalInput")
    qt = nc.dram_tensor("qt", [D, H], BF16, kind="ExternalInput")
    # raw bank layout: (g, 128 rows = [g2 x (7j+h | pad)], half, k, d);
    # host reassembles into (T, H, D)
    ob = nc.dram_tensor("ob", [NG, 128, 2, 2, D], BF16, kind="ExternalOutput")
    with tile.TileContext(nc) as tc:
        with ExitStack() as ctx:
            _body(ctx, tc, xd[:], xjn[:], qt[:], ob[:], repeat=repeat)
    nc.compile()
    return nc


def get_nc(repeat=1):
    key = ("nc", repeat)
    if key not in _CACHE:
        _CACHE[key] = _build(repeat)
    return _CACHE[key]


def prep_inputs(others_b):
    """others[b] (N,T,D) -> (xd, xjn) bf16 layouts.

    xd[d, g, 128c'+64j+n] = others[n, t, d] for t = 2(16g+c')+j
    xjn[64j+n, tp, d] = others[n, 2 tp + j, d], with a trailing ones col.
    """
    xd = np.ascontiguousarray(
        others_b.transpose(2, 1, 0)                 # d, t, n
    ).reshape(128, NG, CH * 128).astype(BF16_NP)
    xjn = np.empty((128, T // 2, D + 1), dtype=BF16_NP)
    v = others_b.reshape(N, T // 2, 2, D)           # n, tp, j, d
    xjn[:, :, D] = 1.0
    xjn[0:64, :, 0:D] = v[:, :, 0, :]
    xjn[64:128, :, 0:D] = v[:, :, 1, :]
    return xd, xjn


def kernel(ego=None, others=None, queries=None, _trace=False, **_unused):
    others = np.asarray(others, dtype=np.float32)
    queries = np.asarray(queries, dtype=np.float32)
    scale = float(queries.shape[-1]) ** -0.5
    qt_scaled = np.ascontiguousarray(queries.T * scale).astype(BF16_NP)

    nc = get_nc()
    in_maps = []
    for b in range(B):
        xd, xjn = prep_inputs(others[b])
        in_maps.append({"xd": xd, "xjn": xjn, "qt": qt_scaled})
    res = run_bass_kernel_spmd(nc, in_maps, core_ids=list(range(B)), trace=_trace)
    _CACHE["last_results"] = res
    out = np.empty((B, T, H, D), dtype=np.float32)
    for b in range(B):
        out[b] = unpack_output(res.results[b]["ob"])
    return out


def unpack_output(ob_raw):
    """(NG, 128, 2, 2, D) layout -> (T, H, D); t = 32 g + 16 half + 4 g2 + 2k + j."""
    s = ob_raw.transpose(0, 2, 1, 3, 4)            # g, half, p, k, d
    s = s.reshape(T // 16, 4, 32, 2, D)[:, :, : 2 * H]
    s = s.reshape(T // 16, 4, 2, H, 2, D)          # ib, g2, j, h, k, d
    return np.ascontiguousarray(
        s.transpose(0, 1, 4, 2, 3, 5).reshape(T, H, D).astype(np.float32)
    )


# revision 13
# speedup vs baseline: 1.0974x; 1.0974x over previous
"""MultiHeadPool Trainium2 kernel (bf16, transpose-free).

Per-core computation (batch b of 8, one per NeuronCore):
  X = others[b]          (N=64, T=512, D=128)
  L = X . qT * scale     contraction over d   -> (T, H, N) logits
  W = softmax_n(L)
  ctx = W . X            contraction over n   -> (T, H, D)

The kernel is PE-instruction-issue bound (~70-80ns/instr), so the design
minimizes Tensor-engine instruction count:
  - bf16 operands: 1 LDWEIGHTS + 1 MATMUL per logical matmul (fp32 emits
    2+2), and half the HBM traffic.
  - The host sends TWO layouts of X: xd[d, (t n)] (d-major) and
    xjn[(j n), tp, d'] (pair-major, d'=D plus a ones column). mm1 loads
    its stationary X^T_c directly from xd -- no PE transposes, no
    PSUM->SBUF relay copies.
  - Block-diagonal zero padding of the E tiles is written ONCE into a
    persistent 2-slot tile; per-chunk exp writes only the diagonal
    blocks.

Per t-pair c (rows = 64j+n, two timesteps t=2c+j):
  mm1: L_c[(jn), h] = xd[:,128c:128c+128].T @ qt      (PSUM, f=7)
  exp: E[(jn), c, j'*7+h] = exp(L) on the j==j' diagonal blocks (bf16)
  mm2: ctx-pair = E_c.T @ xjn_c -> (32, 129) at col-group 32*g2;
       col 128 = softmax denominators; 4 col-groups packed on the PE
  DVE: reciprocal + broadcast scale; DMA out (host reassembles order)
"""

import sys

for p in ("/opt/trn_rl_repo", "/root/.axon_site/_ro/trn_rl_repo"):
    if p not in sys.path:
        sys.path.append(p)

from contextlib import ExitStack

import numpy as np
import ml_dtypes

import concourse.bacc as bacc
import concourse.bass as bass
import concourse.tile as tile
from concourse import mybir
from concourse.bass_utils import run_bass_kernel_spmd

B, N, T, D, H = 8, 64, 512, 128, 7
CH = 16               # t-pairs per chunk
NG = (T // 2) // CH   # 16 chunks per batch
F32 = mybir.dt.float32
BF16 = mybir.dt.bfloat16
BF16_NP = ml_dtypes.bfloat16

_CACHE = {}


def _body(ctx, tc, xd, xjn, qt, ob, repeat=1):
    nc = tc.nc
    E2 = 2 * H            # 14 data cols; pad to 32 for full g2 row groups
    M2 = 32
    DE = D + 1            # 129
    NSLOT = 2             # e-tile slots (manual rotation, zeros persist)

    singles = ctx.enter_context(tc.tile_pool(name="singles", bufs=1))
    ltp = ctx.enter_context(tc.tile_pool(name="ltp", bufs=1, space="PSUM"))
    ctxp = ctx.enter_context(tc.tile_pool(name="ctxp", bufs=3, space="PSUM"))
    rp = ctx.enter_context(tc.tile_pool(name="rp", bufs=4))
    stg = ctx.enter_context(tc.tile_pool(name="stg", bufs=3))

    qt_sb = singles.tile([D, H], BF16)
    nc.sync.dma_start(out=qt_sb[:], in_=qt[:])

    # persistent SBUF-resident inputs; chunk DMAs are issued with a small
    # prefetch depth so output DMAs (scalar queue) interleave with the
    # input stream instead of queueing behind all of it
    xd_sb = singles.tile([128, NG, CH * 128], BF16)
    xjn_sb = singles.tile([128, T // 2, DE], BF16)
    PF = 3

    def fetch(g):
        # xd and xjn on separate DMA dispatch queues (SP / Activation) so
        # the two input streams transfer concurrently
        nc.sync.dma_start(out=xd_sb[:, g, :], in_=xd[:, g, :])
        nc.scalar.dma_start(out=xjn_sb[:, CH * g: CH * (g + 1), :],
                            in_=xjn[:, CH * g: CH * (g + 1), :])

    for g in range(min(PF, NG)):
        fetch(g)

    # E tiles: [jn, slot, c, m]; m in 0:7 is the j=0 block, 7:14 the j=1
    # block, 14:32 zero pad (keeps mm2 output a full 32-row group). The
    # off-diagonal + pad zeros are written once and never touched again.
    e_all = singles.tile([128, NSLOT, CH, M2], BF16)
    nc.vector.memset(e_all[:], 0.0)

    # one persistent L bank with 4 rotating slots (448 fp32 <= 1 bank);
    # slot-reuse deps are 4 chunks old -> elided
    ltall = ltp.tile([128, 4, CH, H], F32)

    for g in range(NG * repeat):
        g = g % NG
        if g + PF < NG:
            fetch(g + PF)
        ltb = ltall[:, g % 4]
        for i in range(CH):
            nc.tensor.matmul(
                ltb[:, i, :],
                lhsT=xd_sb[:, g, 128 * i: 128 * (i + 1)],
                rhs=qt_sb[:],
                start=True, stop=True,
            )

        e_g = e_all[:, g % NSLOT]
        nc.scalar.activation(
            out=e_g[0:64, :, 0:H], in_=ltb[0:64],
            func=mybir.ActivationFunctionType.Exp,
        )
        nc.scalar.activation(
            out=e_g[64:128, :, H:E2], in_=ltb[64:128],
            func=mybir.ActivationFunctionType.Exp,
        )

        # mm2: 8 pairs per PSUM bank; col-groups iterate fastest so the
        # four 32-col PE tiles run concurrently
        stc = stg.tile([128, 2, 2, D], BF16)
        for half in range(2):
            ctxb = ctxp.tile([128, 2, DE], F32)
            for k in range(2):
                for g2 in range(4):
                    c = half * 8 + 2 * g2 + k
                    nc.tensor.matmul(
                        ctxb[32 * g2: 32 * (g2 + 1), k, :],
                        lhsT=e_g[:, c, :],
                        rhs=xjn_sb[:, CH * g + c, :],
                        start=True, stop=True,
                        tile_position=(0, 32 * g2),
                    )
            rr = rp.tile([128, 2, 1], F32)
            nc.vector.reciprocal(rr[:, :, 0], ctxb[:, :, D])
            nc.vector.tensor_mul(
                stc[:, half], ctxb[:, :, 0:D], rr[:].to_broadcast([128, 2, D]),
            )
        # one store per chunk on the gpsimd (SWDGE) queue, independent of
        # both input streams
        nc.gpsimd.dma_start(out=ob[g], in_=stc[:])


def _build(repeat=1):
    # Bacc (not bare Bass): its compile() runs move_matmul_waits_to_ldweights
    # + generate_event_semaphores, which legalize multi-wait instructions for
    # the TRN2 one-wait-per-instruction constraint.
    nc = bacc.Bacc("TRN2", target_bir_lowering=False, debug=False)
    xd = nc.dram_tensor("xd", [128, NG, CH * 128], BF16, kind="ExternalInput")
    xjn = nc.dram_tensor("xjn", [128, T // 2, D + 1], BF16, kind="ExternalInput")
    qt = nc.dram_tensor("qt", [D, H], BF16, kind="ExternalInput")
    # raw bank layout: (g, 128 rows = [g2 x (7j+h | pad)], half, k, d);
    # host reassembles into (T, H, D)
    ob = nc.dram_tensor("ob", [NG, 128, 2, 2, D], BF16, kind="ExternalOutput")
    with tile.TileContext(nc) as tc:
        with ExitStack() as ctx:
            _body(ctx, tc, xd[:], xjn[:], qt[:], ob[:], repeat=repeat)
    nc.compile()
    return nc


def get_nc(repeat=1):
    key = ("nc", repeat)
    if key not in _CACHE:
        _CACHE[key] = _build(repeat)
    return _CACHE[key]


def prep_inputs(others_b):
    """others[b] (N,T,D) -> (xd, xjn) bf16 layouts.

    xd[d, g, 128c'+64j+n] = others[n, t, d] for t = 2(16g+c')+j
    xjn[64j+n, tp, d] = others[n, 2 tp + j, d], with a trailing ones col.
    """
    xd = np.ascontiguousarray(
        others_b.transpose(2, 1, 0)                 # d, t, n
    ).reshape(128, NG, CH * 128).astype(BF16_NP)
    xjn = np.empty((128, T // 2, D + 1), dtype=BF16_NP)
    v = others_b.reshape(N, T // 2, 2, D)           # n, tp, j, d
    xjn[:, :, D] = 1.0
    xjn[0:64, :, 0:D] = v[:, :, 0, :]
    xjn[64:128, :, 0:D] = v[:, :, 1, :]
    return xd, xjn


def kernel(ego=None, others=None, queries=None, _trace=False, **_unused):
    others = np.asarray(others, dtype=np.float32)
    queries = np.asarray(queries, dtype=np.float32)
    scale = float(queries.shape[-1]) ** -0.5
    qt_scaled = np.ascontiguousarray(queries.T * scale).astype(BF16_NP)

    nc = get_nc()
    in_maps = []
    for b in range(B):
        xd, xjn = prep_inputs(others[b])
        in_maps.append({"xd": xd, "xjn": xjn, "qt": qt_scaled})
    res = run_bass_kernel_spmd(nc, in_maps, core_ids=list(range(B)), trace=_trace)
    _CACHE["last_results"] = res
    out = np.empty((B, T, H, D), dtype=np.float32)
    for b in range(B):
        out[b] = unpack_output(res.results[b]["ob"])
    return out


def unpack_output(ob_raw):
    """(NG, 128, 2, 2, D) layout -> (T, H, D); t = 32 g + 16 half + 4 g2 + 2k + j."""
    s = ob_raw.transpose(0, 2, 1, 3, 4)            # g, half, p, k, d
    s = s.reshape(T // 16, 4, 32, 2, D)[:, :, : 2 * H]
    s = s.reshape(T // 16, 4, 2, H, 2, D)          # ib, g2, j, h, k, d
    return np.ascontiguousarray(
        s.transpose(0, 1, 4, 2, 3, 5).reshape(T, H, D).astype(np.float32)
    )


# revision 21
# speedup vs baseline: 1.2344x; 1.1248x over previous
"""MultiHeadPool Trainium2 kernel (bf16, transpose-free).

Per-core computation (batch b of 8, one per NeuronCore):
  X = others[b]          (N=64, T=512, D=128)
  L = X . qT * scale     contraction over d   -> (T, H, N) logits
  W = softmax_n(L)
  ctx = W . X            contraction over n   -> (T, H, D)

The kernel is PE-instruction-issue bound (~70-80ns/instr), so the design
minimizes Tensor-engine instruction count:
  - bf16 operands: 1 LDWEIGHTS + 1 MATMUL per logical matmul (fp32 emits
    2+2), and half the HBM traffic.
  - The host sends TWO layouts of X: xd[d, (t n)] (d-major) and
    xjn[(j n), tp, d'] (pair-major, d'=D plus a ones column). mm1 loads
    its stationary X^T_c directly from xd -- no PE transposes, no
    PSUM->SBUF relay copies.
  - Block-diagonal zero padding of the E tiles is written ONCE into a
    persistent 2-slot tile; per-chunk exp writes only the diagonal
    blocks.

Per t-pair c (rows = 64j+n, two timesteps t=2c+j):
  mm1: L_c[(jn), h] = xd[:,128c:128c+128].T @ qt      (PSUM, f=7)
  exp: E[(jn), c, j'*7+h] = exp(L) on the j==j' diagonal blocks (bf16)
  mm2: ctx-pair = E_c.T @ xjn_c -> (32, 129) at col-group 32*g2;
       col 128 = softmax denominators; 4 col-groups packed on the PE
  DVE: reciprocal + broadcast scale; DMA out (host reassembles order)
"""

import sys

for p in ("/opt/trn_rl_repo", "/root/.axon_site/_ro/trn_rl_repo"):
    if p not in sys.path:
        sys.path.append(p)

from contextlib import ExitStack

import numpy as np
import ml_dtypes

import concourse.bacc as bacc
import concourse.bass as bass
import concourse.tile as tile
from concourse import mybir
from concourse.bass_utils import run_bass_kernel_spmd

B, N, T, D, H = 8, 64, 512, 128, 7
CH = 16               # t-pairs per chunk
NG = (T // 2) // CH   # 16 chunks per batch
F32 = mybir.dt.float32
BF16 = mybir.dt.bfloat16
BF16_NP = ml_dtypes.bfloat16

_CACHE = {}


def _body(ctx, tc, xd, xjn, qt, ob, repeat=1):
    nc = tc.nc
    E2 = 2 * H            # 14 data cols; pad to 32 for full g2 row groups
    M2 = 32
    DE = D + 1            # 129
    NSLOT = 4             # e-tile slots (manual rotation, zeros persist)
    FG = 2                # chunks per DMA (input fetch + output store)

    singles = ctx.enter_context(tc.tile_pool(name="singles", bufs=1))
    ltp = ctx.enter_context(tc.tile_pool(name="ltp", bufs=1, space="PSUM"))
    ctxp = ctx.enter_context(tc.tile_pool(name="ctxp", bufs=4, space="PSUM"))
    rp = ctx.enter_context(tc.tile_pool(name="rp", bufs=4))
    stg = ctx.enter_context(tc.tile_pool(name="stg", bufs=3))

    qt_sb = singles.tile([D, H], BF16)
    nc.sync.dma_start(out=qt_sb[:], in_=qt[:])

    # persistent SBUF-resident inputs; chunk DMAs are issued with a small
    # prefetch depth so output DMAs (scalar queue) interleave with the
    # input stream instead of queueing behind all of it
    xd_sb = singles.tile([128, NG, CH * 128], BF16)
    xjn_sb = singles.tile([128, T // 2, DE], BF16)
    NP = NG // FG         # fetch groups
    PF = 2                # fetch groups of lookahead

    def fetch(p):
        # xd and xjn on separate DMA dispatch queues (SP / Activation) so
        # the two input streams transfer concurrently
        nc.sync.dma_start(out=xd_sb[:, FG * p: FG * (p + 1), :],
                          in_=xd[:, FG * p: FG * (p + 1), :])
        nc.scalar.dma_start(
            out=xjn_sb[:, CH * FG * p: CH * FG * (p + 1), :],
            in_=xjn[:, CH * FG * p: CH * FG * (p + 1), :])

    for p in range(min(PF, NP)):
        fetch(p)

    # E tiles: [jn, slot, c, m]; m in 0:7 is the j=0 block, 7:14 the j=1
    # block, 14:32 zero pad (keeps mm2 output a full 32-row group). The
    # off-diagonal + pad zeros are written once and never touched again.
    e_all = singles.tile([128, NSLOT, CH, M2], BF16)
    nc.vector.memset(e_all[:], 0.0)

    # one persistent L bank with 4 rotating slots (448 fp32 <= 1 bank);
    # slot-reuse deps are 4 chunks old -> elided
    ltall = ltp.tile([128, 4, CH, H], F32)

    for g in range(NG * repeat):
        g = g % NG
        if g % FG == 0 and g // FG + PF < NP:
            fetch(g // FG + PF)
        ltb = ltall[:, g % 4]
        for i in range(CH):
            nc.tensor.matmul(
                ltb[:, i, :],
                lhsT=xd_sb[:, g, 128 * i: 128 * (i + 1)],
                rhs=qt_sb[:],
                start=True, stop=True,
            )

        e_g = e_all[:, g % NSLOT]
        nc.scalar.activation(
            out=e_g[0:64, :, 0:H], in_=ltb[0:64],
            func=mybir.ActivationFunctionType.Exp,
        )
        nc.scalar.activation(
            out=e_g[64:128, :, H:E2], in_=ltb[64:128],
            func=mybir.ActivationFunctionType.Exp,
        )

        # mm2: 8 pairs per PSUM bank; col-groups iterate fastest so the
        # four 32-col PE tiles run concurrently
        if g % FG == 0:
            stc_fg = stg.tile([128, FG, 2, 2, D], BF16)
        stc = stc_fg[:, g % FG]
        for half in range(2):
            ctxb = ctxp.tile([128, 2, DE], F32)
            for k in range(2):
                for g2 in range(4):
                    c = half * 8 + 2 * g2 + k
                    nc.tensor.matmul(
                        ctxb[32 * g2: 32 * (g2 + 1), k, :],
                        lhsT=e_g[:, c, :],
                        rhs=xjn_sb[:, CH * g + c, :],
                        start=True, stop=True,
                        tile_position=(0, 32 * g2),
                    )
            rr = rp.tile([128, 2, 1], F32)
            nc.vector.reciprocal(rr[:, :, 0], ctxb[:, :, D])
            nc.vector.tensor_mul(
                stc[:, half], ctxb[:, :, 0:D], rr[:].to_broadcast([128, 2, D]),
            )
        # one store per fetch group on the gpsimd (SWDGE) queue,
        # independent of both input streams
        if g % FG == FG - 1:
            nc.gpsimd.dma_start(out=ob[g // FG], in_=stc_fg[:])


def _build(repeat=1):
    # Bacc (not bare Bass): its compile() runs move_matmul_waits_to_ldweights
    # + generate_event_semaphores, which legalize multi-wait instructions for
    # the TRN2 one-wait-per-instruction constraint.
    nc = bacc.Bacc("TRN2", target_bir_lowering=False, debug=False)
    xd = nc.dram_tensor("xd", [128, NG, CH * 128], BF16, kind="ExternalInput")
    xjn = nc.dram_tensor("xjn", [128, T // 2, D + 1], BF16, kind="ExternalInput")
    qt = nc.dram_tensor("qt", [D, H], BF16, kind="ExternalInput")
    # raw bank layout: (p, 128 rows = [g2 x (7j+h | pad)], chunk, half, k, d);
    # host reassembles into (T, H, D)
    ob = nc.dram_tensor("ob", [NG // 2, 128, 2, 2, 2, D], BF16,
                        kind="ExternalOutput")
    with tile.TileContext(nc) as tc:
        with ExitStack() as ctx:
            _body(ctx, tc, xd[:], xjn[:], qt[:], ob[:], repeat=repeat)
    nc.compile()
    return nc


def get_nc(repeat=1):
    key = ("nc", repeat)
    if key not in _CACHE:
        _CACHE[key] = _build(repeat)
    return _CACHE[key]


def prep_inputs(others_b):
    """others[b] (N,T,D) -> (xd, xjn) bf16 layouts.

    xd[d, g, 128c'+64j+n] = others[n, t, d] for t = 2(16g+c')+j
    xjn[64j+n, tp, d] = others[n, 2 tp + j, d], with a trailing ones col.
    """
    xd = np.ascontiguousarray(
        others_b.transpose(2, 1, 0)                 # d, t, n
    ).reshape(128, NG, CH * 128).astype(BF16_NP)
    xjn = np.empty((128, T // 2, D + 1), dtype=BF16_NP)
    v = others_b.reshape(N, T // 2, 2, D)           # n, tp, j, d
    xjn[:, :, D] = 1.0
    xjn[0:64, :, 0:D] = v[:, :, 0, :]
    xjn[64:128, :, 0:D] = v[:, :, 1, :]
    return xd, xjn


def kernel(ego=None, others=None, queries=None, _trace=False, **_unused):
    others = np.asarray(others, dtype=np.float32)
    queries = np.asarray(queries, dtype=np.float32)
    scale = float(queries.shape[-1]) ** -0.5
    qt_scaled = np.ascontiguousarray(queries.T * scale).astype(BF16_NP)

    nc = get_nc()
    in_maps = []
    for b in range(B):
        xd, xjn = prep_inputs(others[b])
        in_maps.append({"xd": xd, "xjn": xjn, "qt": qt_scaled})
    res = run_bass_kernel_spmd(nc, in_maps, core_ids=list(range(B)), trace=_trace)
    _CACHE["last_results"] = res
    out = np.empty((B, T, H, D), dtype=np.float32)
    for b in range(B):
        out[b] = unpack_output(res.results[b]["ob"])
    return out


def unpack_output(ob_raw):
    """(NG/2, 128, cp, half, k, D) layout -> (T, H, D)."""
    s = ob_raw.transpose(0, 2, 3, 1, 4, 5)         # p, cp, half, part, k, d
    s = s.reshape(T // 16, 4, 32, 2, D)[:, :, : 2 * H]
    s = s.reshape(T // 16, 4, 2, H, 2, D)          # ib, g2, j, h, k, d
    return np.ascontiguousarray(
        s.transpose(0, 1, 4, 2, 3, 5).reshape(T, H, D).astype(np.float32)
    )


# revision 29
# speedup vs baseline: 1.2880x; 1.0434x over previous
"""MultiHeadPool Trainium2 kernel (bf16, transpose-free).

Per-core computation (batch b of 8, one per NeuronCore):
  X = others[b]          (N=64, T=512, D=128)
  L = X . qT * scale     contraction over d   -> (T, H, N) logits
  W = softmax_n(L)
  ctx = W . X            contraction over n   -> (T, H, D)

The kernel is PE-instruction-issue bound (~70-80ns/instr), so the design
minimizes Tensor-engine instruction count:
  - bf16 operands: 1 LDWEIGHTS + 1 MATMUL per logical matmul (fp32 emits
    2+2), and half the HBM traffic.
  - The host sends TWO layouts of X: xd[d, (t n)] (d-major) and
    xjn[(j n), tp, d'] (pair-major, d'=D plus a ones column). mm1 loads
    its stationary X^T_c directly from xd -- no PE transposes, no
    PSUM->SBUF relay copies.
  - Block-diagonal zero padding of the E tiles is written ONCE into a
    persistent 2-slot tile; per-chunk exp writes only the diagonal
    blocks.

Per t-pair c (rows = 64j+n, two timesteps t=2c+j):
  mm1: L_c[(jn), h] = xd[:,128c:128c+128].T @ qt      (PSUM, f=7)
  exp: E[(jn), c, j'*7+h] = exp(L) on the j==j' diagonal blocks (bf16)
  mm2: ctx-pair = E_c.T @ xjn_c -> (32, 129) at col-group 32*g2;
       col 128 = softmax denominators; 4 col-groups packed on the PE
  DVE: reciprocal + broadcast scale; DMA out (host reassembles order)
"""

import sys

for p in ("/opt/trn_rl_repo", "/root/.axon_site/_ro/trn_rl_repo"):
    if p not in sys.path:
        sys.path.append(p)

from contextlib import ExitStack

import numpy as np
import ml_dtypes

import concourse.bacc as bacc
import concourse.bass as bass
import concourse.tile as tile
from concourse import mybir
from concourse.bass_utils import run_bass_kernel_spmd

B, N, T, D, H = 8, 64, 512, 128, 7
CH = 16               # t-pairs per chunk
NG = (T // 2) // CH   # 16 chunks per batch
F32 = mybir.dt.float32
BF16 = mybir.dt.bfloat16
BF16_NP = ml_dtypes.bfloat16

_CACHE = {}


def _body(ctx, tc, xd, xjn, qt, ob, repeat=1):
    nc = tc.nc
    E2 = 2 * H            # 14 data cols; pad to 32 for full g2 row groups
    M2 = 32
    DE = D + 1            # 129
    NSLOT = 4             # e-tile slots (manual rotation, zeros persist)
    # chunks per DMA group: big groups amortize ring + teardown overhead,
    # small final groups shorten the last-data -> last-output tail
    GROUPS = [4, 4, 4, 2, 1, 1]
    G_OFF = [sum(GROUPS[:i]) for i in range(len(GROUPS) + 1)]
    assert G_OFF[-1] == NG

    singles = ctx.enter_context(tc.tile_pool(name="singles", bufs=1))
    ltp = ctx.enter_context(tc.tile_pool(name="ltp", bufs=1, space="PSUM"))
    ctxp = ctx.enter_context(tc.tile_pool(name="ctxp", bufs=4, space="PSUM"))
    rp = ctx.enter_context(tc.tile_pool(name="rp", bufs=4))
    stg = ctx.enter_context(tc.tile_pool(name="stg", bufs=3))

    # persistent SBUF-resident inputs; chunk DMAs are issued with a small
    # prefetch depth so output DMAs interleave with the input stream
    # instead of queueing behind all of it
    qt_sb = singles.tile([D, H], BF16)
    xd_sb = singles.tile([128, NG, CH * 128], BF16)
    xjn_sb = singles.tile([128, T // 2, DE], BF16)
    NP = len(GROUPS)      # fetch groups
    PF = 2                # fetch groups of lookahead

    def fetch(p):
        # xd and xjn on separate DMA dispatch queues (SP / Activation) so
        # the two input streams transfer concurrently
        g0, g1 = G_OFF[p], G_OFF[p + 1]
        nc.sync.dma_start(out=xd_sb[:, g0:g1, :], in_=xd[:, g0:g1, :])
        nc.scalar.dma_start(out=xjn_sb[:, CH * g0: CH * g1, :],
                            in_=xjn[:, CH * g0: CH * g1, :])

    nc.sync.dma_start(out=qt_sb[:], in_=qt[:])
    for p in range(min(PF, NP)):
        fetch(p)

    # E tiles: [jn, slot, c, m]; m in 0:7 is the j=0 block, 7:14 the j=1
    # block, 14:32 zero pad (keeps mm2 output a full 32-row group). The
    # off-diagonal + pad zeros are written once and never touched again.
    e_all = singles.tile([128, NSLOT, CH, M2], BF16)
    nc.vector.memset(e_all[:], 0.0)

    # one persistent L bank with 4 rotating slots (448 fp32 <= 1 bank);
    # slot-reuse deps are 4 chunks old -> elided
    ltall = ltp.tile([128, 4, CH, H], F32)

    for g in range(NG * repeat):
        g = g % NG
        p = next(i for i in range(NP) if G_OFF[i] <= g < G_OFF[i + 1])
        if g == G_OFF[p] and p + PF < NP:
            fetch(p + PF)
        ltb = ltall[:, g % 4]
        for i in range(CH):
            nc.tensor.matmul(
                ltb[:, i, :],
                lhsT=xd_sb[:, g, 128 * i: 128 * (i + 1)],
                rhs=qt_sb[:],
                start=True, stop=True,
            )

        e_g = e_all[:, g % NSLOT]
        nc.scalar.activation(
            out=e_g[0:64, :, 0:H], in_=ltb[0:64],
            func=mybir.ActivationFunctionType.Exp,
        )
        nc.scalar.activation(
            out=e_g[64:128, :, H:E2], in_=ltb[64:128],
            func=mybir.ActivationFunctionType.Exp,
        )

        # mm2: 8 pairs per PSUM bank; col-groups iterate fastest so the
        # four 32-col PE tiles run concurrently
        if g == G_OFF[p]:
            stc_fg = stg.tile([128, GROUPS[p], 2, 2, D], BF16)
        stc = stc_fg[:, g - G_OFF[p]]
        for half in range(2):
            ctxb = ctxp.tile([128, 2, DE], F32)
            for k in range(2):
                for g2 in range(4):
                    c = half * 8 + 2 * g2 + k
                    nc.tensor.matmul(
                        ctxb[32 * g2: 32 * (g2 + 1), k, :],
                        lhsT=e_g[:, c, :],
                        rhs=xjn_sb[:, CH * g + c, :],
                        start=True, stop=True,
                        tile_position=(0, 32 * g2),
                    )
            rr = rp.tile([128, 2, 1], F32)
            nc.vector.reciprocal(rr[:, :, 0], ctxb[:, :, D])
            nc.vector.tensor_mul(
                stc[:, half], ctxb[:, :, 0:D], rr[:].to_broadcast([128, 2, D]),
            )
        # one store per fetch group; early groups ride the gpsimd (SWDGE)
        # queue so they never block the input streams, the last two ride
        # the by-then-idle input rings (much faster than SWDGE)
        if g == G_OFF[p + 1] - 1:
            out_ap = ob[:, G_OFF[p]: G_OFF[p + 1]]
            eng = (nc.gpsimd if p < NP - 2
                   else (nc.scalar if p == NP - 2 else nc.sync))
            eng.dma_start(out=out_ap, in_=stc_fg[:])


def _build(repeat=1):
    # Bacc (not bare Bass): its compile() runs move_matmul_waits_to_ldweights
    # + generate_event_semaphores, which legalize multi-wait instructions for
    # the TRN2 one-wait-per-instruction constraint.
    nc = bacc.Bacc("TRN2", target_bir_lowering=False, debug=False)
    xd = nc.dram_tensor("xd", [128, NG, CH * 128], BF16, kind="ExternalInput")
    xjn = nc.dram_tensor("xjn", [128, T // 2, D + 1], BF16, kind="ExternalInput")
    qt = nc.dram_tensor("qt", [D, H], BF16, kind="ExternalInput")
    # raw bank layout: (128 rows = [g2 x (7j+h | pad)], chunk, half, k, d);
    # host reassembles into (T, H, D)
    ob = nc.dram_tensor("ob", [128, NG, 2, 2, D], BF16,
                        kind="ExternalOutput")
    with tile.TileContext(nc) as tc:
        with ExitStack() as ctx:
            _body(ctx, tc, xd[:], xjn[:], qt[:], ob[:], repeat=repeat)
    nc.compile()
    return nc


def get_nc(repeat=1):
    key = ("nc", repeat)
    if key not in _CACHE:
        _CACHE[key] = _build(repeat)
    return _CACHE[key]


def prep_inputs(others_b):
    """others[b] (N,T,D) -> (xd, xjn) bf16 layouts.

    xd[d, g, 128c'+64j+n] = others[n, t, d] for t = 2(16g+c')+j
    xjn[64j+n, tp, d] = others[n, 2 tp + j, d], with a trailing ones col.
    """
    xd = np.ascontiguousarray(
        others_b.transpose(2, 1, 0)                 # d, t, n
    ).reshape(128, NG, CH * 128).astype(BF16_NP)
    xjn = np.empty((128, T // 2, D + 1), dtype=BF16_NP)
    v = others_b.reshape(N, T // 2, 2, D)           # n, tp, j, d
    xjn[:, :, D] = 1.0
    xjn[0:64, :, 0:D] = v[:, :, 0, :]
    xjn[64:128, :, 0:D] = v[:, :, 1, :]
    return xd, xjn


def kernel(ego=None, others=None, queries=None, _trace=False, **_unused):
    others = np.asarray(others, dtype=np.float32)
    queries = np.asarray(queries, dtype=np.float32)
    scale = float(queries.shape[-1]) ** -0.5
    qt_scaled = np.ascontiguousarray(queries.T * scale).astype(BF16_NP)

    nc = get_nc()
    in_maps = []
    for b in range(B):
        xd, xjn = prep_inputs(others[b])
        in_maps.append({"xd": xd, "xjn": xjn, "qt": qt_scaled})
    res = run_bass_kernel_spmd(nc, in_maps, core_ids=list(range(B)), trace=_trace)
    _CACHE["last_results"] = res
    out = np.empty((B, T, H, D), dtype=np.float32)
    for b in range(B):
        out[b] = unpack_output(res.results[b]["ob"])
    return out


def unpack_output(ob_raw):
    """(128, NG, half, k, D) layout -> (T, H, D)."""
    s = ob_raw.transpose(1, 2, 0, 3, 4)            # g, half, part, k, d
    s = s.reshape(T // 16, 4, 32, 2, D)[:, :, : 2 * H]
    s = s.reshape(T // 16, 4, 2, H, 2, D)          # ib, g2, j, h, k, d
    return np.ascontiguousarray(
        s.transpose(0, 1, 4, 2, 3, 5).reshape(T, H, D).astype(np.float32)
    )
